# revision 11
# baseline (speedup 1.0000x reference)
"""DiscreteBipartiteFlow forward on 8 Trainium2 NeuronCores.

Math (forward pass only):
  masked = mask * inputs                      (mask = 1 at odd l, 0 at even l)
  h   = relu(masked.reshape(B, L*V) @ W1 + b1)
  net = (h @ W2 + b2).reshape(B, L, 2V)
  loc, scale = argmax one-hots of net[..., :V], net[..., V:]
  out[odd l]  = inputs
  out[even l] = onehot((inv(scale) * ((tok - loc) mod V)) mod V), or 0 if scale==0

st_one_hot_argmax's forward value is exactly the hard one-hot (soft terms
cancel), so the post-MLP flow is pure index arithmetic mod 23.

Sharding (8 cores):
  mm1: tensor-parallel over hidden. Core k computes hT[512k:512k+512, :] from
       the odd-position one-hot rows only (mask zeroes even rows; 2944 of 5888
       W1 rows ever contribute).
  all-gather: h, split into bf16 hi+lo, gathered in 4 chunks (one per local
       128-row tile) so communication pipelines under mm2; a zero-dep dummy
       collective at t=0 absorbs cross-core start skew + comm setup.
  mm2: tensor-parallel over output columns. Core k owns positions
       l in [32k, 32k+32) = 1472 columns of W2. The contraction loop is
       grouped by gather chunk (j % 4) so each chunk's matmuls start as soon
       as its gather lands.
  epilogue: per-core argmax + modular flow for its 16 even positions;
       host interleaves position slices and passes odd positions through.

Precision: matmuls run as bf16 hi/lo split passes (x one-hot is exact in
bf16, so mm1 = 2 passes over W1{hi,lo}; mm2 = 3 passes hh+hl+lh) with fp32
PSUM accumulation -> ~2^-18 operand error, fp32-grade argmax fidelity, at
1 cycle/row TensorE throughput (vs 4 cycles/row for native fp32).
"""

import numpy as np
import ml_dtypes

B, L, V = 512, 256, 23
H = 4096
NCORES = 8
HS = H // NCORES          # 512  hidden shard
HM = HS // 128            # 4    local hidden tiles
PS = L // NCORES          # 32   positions per core
EP = PS // 2              # 16   even positions per core
CW = PS * 2 * V           # 1472 net columns per core
NCH = 4                   # column chunks for mm2
CC = CW // NCH            # 368  columns per chunk (8 positions)
KT1 = (L // 2) * V // 128  # 23 contraction tiles for mm1
KT2 = H // 128            # 32 contraction tiles for mm2
MT = B // 128             # 4 batch tiles

BIG = 64.0
MAGIC = 12582912.0        # 1.5 * 2^23: float32 round-to-int domain
BF16 = ml_dtypes.bfloat16

_cache = {}


def _build():
    import concourse.mybir as mybir
    import concourse.tile as tile
    from concourse import bacc

    fp32 = mybir.dt.float32
    bf16 = mybir.dt.bfloat16
    Alu = mybir.AluOpType
    Act = mybir.ActivationFunctionType

    nc = bacc.Bacc("TRN2", target_bir_lowering=False, debug=False,
                   num_devices=NCORES)

    # ---- per-core inputs ----
    xt = nc.dram_tensor("xt", [KT1, 128, B], bf16, kind="ExternalInput")
    w1h = nc.dram_tensor("w1h", [KT1, 128, HS], bf16, kind="ExternalInput")
    w1l = nc.dram_tensor("w1l", [KT1, 128, HS], bf16, kind="ExternalInput")
    b1s = nc.dram_tensor("b1s", [HM, 128], fp32, kind="ExternalInput")
    # W2 pre-tiled on host: [j, nch, 128, CC] so stream tiles are contiguous
    w2h = nc.dram_tensor("w2h", [KT2, NCH, 128, CC], bf16, kind="ExternalInput")
    w2l = nc.dram_tensor("w2l", [KT2, NCH, 128, CC], bf16, kind="ExternalInput")
    b2r = nc.dram_tensor("b2r", [128, CW], fp32, kind="ExternalInput")
    inpe = nc.dram_tensor("inpe", [MT, 128, EP * V], fp32, kind="ExternalInput")
    oute = nc.dram_tensor("oute", [MT, 128, EP * V], fp32, kind="ExternalOutput")

    # ---- constants (baked into the NEFF) ----
    iota_np = np.arange(V, dtype=np.float32)[None, :].repeat(128, 0)
    c_iota = nc.inline_tensor(np.ascontiguousarray(iota_np), name="c_iota")
    c_bi = nc.inline_tensor(np.ascontiguousarray(BIG - iota_np), name="c_bi")

    with tile.TileContext(nc) as tc:
        with (
            tc.tile_pool(name="persist", bufs=1) as persist,
            tc.tile_pool(name="hwork", bufs=2) as hwork,
            tc.tile_pool(name="w2s", bufs=3) as w2s,
            tc.tile_pool(name="ep", bufs=2) as ep,
            tc.tile_pool(name="small", bufs=2) as small,
            tc.tile_pool(name="ps", bufs=1, space="PSUM") as ps,
            tc.tile_pool(name="dram", bufs=1, space="DRAM") as dram,
        ):
            # ---------- dummy collective: absorbs start skew + comm setup ---
            warm_in = dram.tile([1, 16], fp32, tag="warm_in")
            warm_out = dram.tile([NCORES, 16], fp32, tag="warm_out",
                                 addr_space="Shared")
            nc.gpsimd.collective_compute(
                "AllGather", Alu.bypass,
                replica_groups=[list(range(NCORES))],
                ins=[warm_in.opt()], outs=[warm_out.opt()],
            )

            # ---------- constants to SBUF ----------
            iota_t = persist.tile([128, V], fp32, tag="iota")
            nc.sync.dma_start(iota_t[:], c_iota[:])
            cbi_t = persist.tile([128, V], fp32, tag="cbi")
            nc.sync.dma_start(cbi_t[:], c_bi[:])
            b2_t = persist.tile([128, CW], fp32, tag="b2")
            nc.sync.dma_start(b2_t[:], b2r[:])

            # ---------- load mm1 operands ----------
            xt_t, w1h_t, w1l_t = [], [], []
            for k in range(KT1):
                t = persist.tile([128, B], bf16, tag=f"xt{k}")
                nc.sync.dma_start(t[:], xt[k])
                xt_t.append(t)
                th = persist.tile([128, HS], bf16, tag=f"w1h{k}")
                nc.sync.dma_start(th[:], w1h[k])
                w1h_t.append(th)
                tl = persist.tile([128, HS], bf16, tag=f"w1l{k}")
                nc.sync.dma_start(tl[:], w1l[k])
                w1l_t.append(tl)
            b1_t = []
            for m in range(HM):
                t = persist.tile([128, 1], fp32, tag=f"b1{m}")
                nc.sync.dma_start(t[:], b1s[m].unsqueeze(1))
                b1_t.append(t)

            # token index per batch tile (dep-free, runs during mm1)
            t_tok = []
            for m in range(MT):
                it = ep.tile([128, EP * V], fp32, tag="inpe")
                nc.sync.dma_start(it[:], inpe[m])
                tk = persist.tile([128, EP], fp32, tag=f"tok{m}")
                tmp = ep.tile([128, EP, V], fp32, tag="tokmul")
                nc.vector.tensor_tensor(
                    tmp[:], it[:].rearrange("p (e v) -> p e v", v=V),
                    iota_t[:].unsqueeze(1).broadcast_to([128, EP, V]), Alu.mult)
                nc.vector.tensor_reduce(tk[:], tmp[:], axis=mybir.AxisListType.X,
                                        op=Alu.add)
                t_tok.append(tk)

            # ---------- per-chunk collective buffers ----------
            ag_in = [dram.tile([2, 128, B], bf16, tag=f"ag_in{m}",
                               name=f"ag_in{m}") for m in range(HM)]
            ag_out = [dram.tile([NCORES, 2, 128, B], bf16, tag=f"ag_out{m}",
                                name=f"ag_out{m}", addr_space="Shared")
                      for m in range(HM)]

            # ---------- phase 1: mm1 -> local hT tile, relu, split, gather --
            hhi_loc, hlo_loc = [], []
            for m in range(HM):
                acc = ps.tile([128, B], fp32, tag=f"p4_0_{m}", name=f"ps1_{m}")
                for k in range(KT1):
                    nc.tensor.matmul(acc[:], w1h_t[k][:, m * 128:(m + 1) * 128],
                                     xt_t[k][:], start=(k == 0), stop=False)
                for k in range(KT1):
                    nc.tensor.matmul(acc[:], w1l_t[k][:, m * 128:(m + 1) * 128],
                                     xt_t[k][:], start=False, stop=(k == KT1 - 1))
                hf = hwork.tile([128, B], fp32, tag="hf")
                nc.scalar.activation(hf[:], acc[:], Act.Relu, bias=b1_t[m], scale=1.0)
                hhi = hwork.tile([128, B], bf16, tag=f"hhi{m}", bufs=1)
                nc.vector.tensor_copy(hhi[:], hf[:])
                hlo = hwork.tile([128, B], bf16, tag=f"hlo{m}", bufs=1)
                nc.vector.tensor_sub(hlo[:], hf[:], hhi[:])
                hhi_loc.append(hhi)
                hlo_loc.append(hlo)
                nc.sync.dma_start(ag_in[m][0], hhi[:])
                nc.sync.dma_start(ag_in[m][1], hlo[:])
                nc.gpsimd.collective_compute(
                    "AllGather", Alu.bypass,
                    replica_groups=[list(range(NCORES))],
                    ins=[ag_in[m].opt()], outs=[ag_out[m].opt()],
                )

            # ---------- phase 3: gathered hT to SBUF (per chunk) ----------
            # reuse the xt/w1 tile slots -- same size, dead after mm1
            recycle = ([f"xt{k}" for k in range(KT1)]
                       + [f"w1h{k}" for k in range(KT1)]
                       + [f"w1l{k}" for k in range(KT1)])
            hth_t, htl_t = [None] * KT2, [None] * KT2
            for g in range(HM):
                for s in range(NCORES):
                    j = HM * s + g
                    th = persist.tile([128, B], bf16, tag=recycle[2 * j],
                                      name=f"hth{j}")
                    nc.sync.dma_start(th[:], ag_out[g][s, 0])
                    hth_t[j] = th
                    tl = persist.tile([128, B], bf16, tag=recycle[2 * j + 1],
                                      name=f"htl{j}")
                    nc.sync.dma_start(tl[:], ag_out[g][s, 1])
                    htl_t[j] = tl

            idx_all = [persist.tile([128, EP, 2], fp32, tag=f"idx{m}",
                                    name=f"idx{m}")
                       for m in range(MT)]

            # ---------- phase 4: mm2 --------------------------------------
            # m-tiles in pairs; each (j, m) loads the hT hi/lo stationaries
            # once and streams all 4 column chunks through them
            # (2 LDWEIGHTS per 12 matmuls). W2 is streamed once per m-pair.
            def mm2_epilogue(acc, nch, m):
                pv = acc[:].rearrange("p (i q c) -> p i q c", q=2, c=2 * V)
                bv = b2_t[:, nch * CC:(nch + 1) * CC].rearrange(
                    "p (i q c) -> p i q c", q=2, c=2 * V)
                netE = ep.tile([128, 4, 2 * V], fp32, tag="netE", name="netE")
                nc.vector.tensor_tensor(netE[:], pv[:, :, 0], bv[:, :, 0],
                                        Alu.add)
                ng = netE[:].rearrange("p i (s v) -> p i s v", v=V)
                gmax = ep.tile([128, 4, 2], fp32, tag="gmax", name="gmax")
                nc.vector.tensor_reduce(gmax[:], ng, axis=mybir.AxisListType.X,
                                        op=Alu.max)
                eq = ep.tile([128, 4, 2, V], fp32, tag="eq", name="eq")
                nc.vector.tensor_tensor(
                    eq[:], ng, gmax[:].unsqueeze(3).broadcast_to([128, 4, 2, V]),
                    Alu.is_ge)
                mt = ep.tile([128, 4, 2, V], fp32, tag="mt", name="mt")
                nc.vector.tensor_tensor(
                    mt[:], eq[:],
                    cbi_t[:].unsqueeze(1).unsqueeze(1).broadcast_to(
                        [128, 4, 2, V]), Alu.mult)
                tmax = ep.tile([128, 4, 2], fp32, tag="tmax", name="tmax")
                nc.vector.tensor_reduce(tmax[:], mt[:], axis=mybir.AxisListType.X,
                                        op=Alu.max)
                nc.vector.tensor_scalar(
                    idx_all[m][:, nch * 4:(nch + 1) * 4, :],
                    tmax[:], -1.0, BIG, Alu.mult, Alu.add)

            def mod23(dst_tag, src):
                d = small.tile([128, EP], fp32, tag=dst_tag + "_d",
                               name=dst_tag + "_d")
                nc.vector.tensor_scalar(d[:], src[:], 1.0 / 23.0, -0.49,
                                        Alu.mult, Alu.add)
                q = small.tile([128, EP], fp32, tag=dst_tag + "_q",
                               name=dst_tag + "_q")
                nc.vector.tensor_scalar(q[:], d[:], MAGIC, MAGIC,
                                        Alu.add, Alu.subtract)
                r = small.tile([128, EP], fp32, tag=dst_tag + "_r",
                               name=dst_tag + "_r")
                nc.vector.scalar_tensor_tensor(r[:], q[:], -23.0, src[:],
                                               Alu.mult, Alu.add)
                return r

            def flow_out(m):
                """argmax indices -> modular flow -> one-hot -> DRAM."""
                loc = idx_all[m][:, :, 0]
                scl = idx_all[m][:, :, 1]
                u0 = small.tile([128, EP], fp32, tag="u0", name="u0")
                nc.vector.scalar_tensor_tensor(u0[:], t_tok[m][:], 23.0, loc,
                                               Alu.add, Alu.subtract)
                geu = small.tile([128, EP], fp32, tag="geu", name="geu")
                nc.vector.tensor_single_scalar(geu[:], u0[:], 23.0, Alu.is_ge)
                u = small.tile([128, EP], fp32, tag="u", name="u")
                nc.vector.scalar_tensor_tensor(u[:], geu[:], -23.0, u0[:],
                                               Alu.mult, Alu.add)
                s2 = small.tile([128, EP], fp32, tag="s2", name="s2")
                nc.vector.tensor_tensor(s2[:], scl, scl, Alu.mult)
                s2m = mod23("s2m", s2)
                s4 = small.tile([128, EP], fp32, tag="s4", name="s4")
                nc.vector.tensor_tensor(s4[:], s2m[:], s2m[:], Alu.mult)
                s4m = mod23("s4m", s4)
                s8 = small.tile([128, EP], fp32, tag="s8", name="s8")
                nc.vector.tensor_tensor(s8[:], s4m[:], s4m[:], Alu.mult)
                s8m = mod23("s8m", s8)
                s16 = small.tile([128, EP], fp32, tag="s16", name="s16")
                nc.vector.tensor_tensor(s16[:], s8m[:], s8m[:], Alu.mult)
                s16m = mod23("s16m", s16)
                p1 = small.tile([128, EP], fp32, tag="p1", name="p1")
                nc.vector.tensor_tensor(p1[:], s16m[:], s4m[:], Alu.mult)
                p1m = mod23("p1m", p1)
                p2 = small.tile([128, EP], fp32, tag="p2", name="p2")
                nc.vector.tensor_tensor(p2[:], p1m[:], scl, Alu.mult)
                inv = mod23("inv", p2)
                wprod = small.tile([128, EP], fp32, tag="wprod", name="wprod")
                nc.vector.tensor_tensor(wprod[:], inv[:], u[:], Alu.mult)
                wm = mod23("wm", wprod)
                live = small.tile([128, EP], fp32, tag="live", name="live")
                nc.vector.tensor_single_scalar(live[:], inv[:], 0.5, Alu.is_ge)
                w1p = small.tile([128, EP], fp32, tag="w1p", name="w1p")
                nc.vector.tensor_single_scalar(w1p[:], wm[:], 1.0, Alu.add)
                w2p = small.tile([128, EP], fp32, tag="w2p", name="w2p")
                nc.vector.tensor_tensor(w2p[:], w1p[:], live[:], Alu.mult)
                wfin = small.tile([128, EP], fp32, tag="wfin", name="wfin")
                nc.vector.tensor_single_scalar(wfin[:], w2p[:], -1.0, Alu.add)
                oh = ep.tile([128, EP, V], fp32, tag="oh", name="oh")
                nc.vector.tensor_tensor(
                    oh[:], iota_t[:].unsqueeze(1).broadcast_to([128, EP, V]),
                    wfin[:].unsqueeze(2).broadcast_to([128, EP, V]), Alu.is_equal)
                nc.sync.dma_start(oute[m], oh[:].rearrange("p e v -> p (e v)"))

            for mp in range(MT // 2):          # m-pairs (0,1), (2,3)
                ms = (2 * mp, 2 * mp + 1)
                accs = {(mi, nch): ps.tile([128, CC], fp32,
                                           tag=f"p4_{mi}_{nch}",
                                           name=f"p4_{mp}_{mi}_{nch}")
                        for mi in range(2) for nch in range(NCH)}
                for g in range(HM):
                    for s in range(NCORES):
                        j = HM * s + g
                        first = (g == 0 and s == 0)
                        last = (g == HM - 1 and s == NCORES - 1)
                        rh = []
                        rl = []
                        for nch in range(NCH):
                            th = w2s.tile([128, CC], bf16, tag=f"rh{nch}",
                                          name=f"rh{nch}")
                            nc.sync.dma_start(th[:], w2h[j, nch])
                            rh.append(th)
                            tl = w2s.tile([128, CC], bf16, tag=f"rl{nch}",
                                          name=f"rl{nch}")
                            nc.sync.dma_start(tl[:], w2l[j, nch])
                            rl.append(tl)
                        for mi, m in enumerate(ms):
                            lh = hth_t[j][:, m * 128:(m + 1) * 128]
                            ll = htl_t[j][:, m * 128:(m + 1) * 128]
                            for nch in range(NCH):
                                a = accs[(mi, nch)]
                                nc.tensor.matmul(a[:], lh, rh[nch][:],
                                                 start=first, stop=False)
                                nc.tensor.matmul(a[:], lh, rl[nch][:],
                                                 start=False, stop=False)
                            for nch in range(NCH):
                                a = accs[(mi, nch)]
                                nc.tensor.matmul(a[:], ll, rh[nch][:],
                                                 start=False, stop=last)
                for mi, m in enumerate(ms):
                    for nch in range(NCH):
                        mm2_epilogue(accs[(mi, nch)], nch, m)
                for m in ms:
                    flow_out(m)

    nc.compile()
    return nc


def _split_bf16(a):
    hi = a.astype(BF16)
    lo = (a - hi.astype(np.float32)).astype(BF16)
    return hi, lo


def kernel(inputs, mask, W1, b1, W2, b2):
    from concourse.bass_utils import run_bass_kernel_spmd

    if "nc" not in _cache:
        _cache["nc"] = _build()
    nc = _cache["nc"]

    inputs = np.asarray(inputs, np.float32)
    mask = np.asarray(mask, np.float32)
    W1 = np.asarray(W1, np.float32)
    b1 = np.asarray(b1, np.float32)
    W2 = np.asarray(W2, np.float32)
    b2 = np.asarray(b2, np.float32)

    masked = inputs * mask[None, :, :]                    # [B, L, V]
    x_odd = masked[:, 1::2, :].reshape(B, (L // 2) * V)   # [512, 2944]
    xt_np = np.ascontiguousarray(x_odd.T.reshape(KT1, 128, B)).astype(BF16)
    W1_odd = W1.reshape(L, V, H)[1::2].reshape((L // 2) * V, H)

    in_maps = []
    for k in range(NCORES):
        w1s = W1_odd[:, k * HS:(k + 1) * HS]
        w1hi, w1lo = _split_bf16(w1s)
        w2s = W2[:, k * CW:(k + 1) * CW]
        w2hi, w2lo = _split_bf16(w2s)
        # pre-tile W2 so each (j, nch) stream tile is a contiguous block
        w2hi = np.ascontiguousarray(
            w2hi.reshape(KT2, 128, NCH, CC).transpose(0, 2, 1, 3))
        w2lo = np.ascontiguousarray(
            w2lo.reshape(KT2, 128, NCH, CC).transpose(0, 2, 1, 3))
        b2s = b2[k * CW:(k + 1) * CW]
        cols = slice(32 * k, 32 * k + 32, 2)
        inpe = inputs[:, cols, :].reshape(MT, 128, EP * V)
        in_maps.append({
            "xt": xt_np,
            "w1h": np.ascontiguousarray(w1hi.reshape(KT1, 128, HS)),
            "w1l": np.ascontiguousarray(w1lo.reshape(KT1, 128, HS)),
            "b1s": np.ascontiguousarray(b1[k * HS:(k + 1) * HS].reshape(-1, 128)),
            "w2h": w2hi,
            "w2l": w2lo,
            "b2r": np.ascontiguousarray(np.broadcast_to(b2s, (128, CW))),
            "inpe": np.ascontiguousarray(inpe),
        })

    res = run_bass_kernel_spmd(nc, in_maps, core_ids=list(range(NCORES)))
    _cache["last_result"] = res

    out = np.empty((B, L, V), np.float32)
    out[:, 1::2, :] = masked[:, 1::2, :]
    for k in range(NCORES):
        oe = res.results[k]["oute"].reshape(MT, 128, EP, V)
        out[:, 32 * k:32 * k + 32:2, :] = oe.reshape(B, EP, V)
    return out


# revision 12
# speedup vs baseline: 1.1543x; 1.1543x over previous
"""DiscreteBipartiteFlow forward on 8 Trainium2 NeuronCores.

Math (forward pass only):
  masked = mask * inputs                      (mask = 1 at odd l, 0 at even l)
  h   = relu(masked.reshape(B, L*V) @ W1 + b1)
  net = (h @ W2 + b2).reshape(B, L, 2V)
  loc, scale = argmax one-hots of net[..., :V], net[..., V:]
  out[odd l]  = inputs
  out[even l] = onehot((inv(scale) * ((tok - loc) mod V)) mod V), or 0 if scale==0

st_one_hot_argmax's forward value is exactly the hard one-hot (soft terms
cancel), so the post-MLP flow is pure index arithmetic mod 23.

Sharding (8 cores):
  mm1: tensor-parallel over hidden. Core k computes hT[512k:512k+512, :] from
       the odd-position one-hot rows only (mask zeroes even rows; 2944 of 5888
       W1 rows ever contribute).
  all-gather: h, split into bf16 hi+lo, gathered in 4 chunks (one per local
       128-row tile) so communication pipelines under mm2; a zero-dep dummy
       collective at t=0 absorbs cross-core start skew + comm setup.
  mm2: tensor-parallel over output columns. Core k owns positions
       l in [32k, 32k+32) = 1472 columns of W2. The contraction loop is
       grouped by gather chunk (j % 4) so each chunk's matmuls start as soon
       as its gather lands.
  epilogue: per-core argmax + modular flow for its 16 even positions;
       host interleaves position slices and passes odd positions through.

Precision: matmuls run as bf16 hi/lo split passes (x one-hot is exact in
bf16, so mm1 = 2 passes over W1{hi,lo}; mm2 = 3 passes hh+hl+lh) with fp32
PSUM accumulation -> ~2^-18 operand error, fp32-grade argmax fidelity, at
1 cycle/row TensorE throughput (vs 4 cycles/row for native fp32).
"""

import numpy as np
import ml_dtypes

B, L, V = 512, 256, 23
H = 4096
NCORES = 8
HS = H // NCORES          # 512  hidden shard
HM = HS // 128            # 4    local hidden tiles
PS = L // NCORES          # 32   positions per core
EP = PS // 2              # 16   even positions per core
CW = PS * 2 * V           # 1472 net columns per core
NCH = 4                   # column chunks for mm2
CC = CW // NCH            # 368  columns per chunk (8 positions)
KT1 = (L // 2) * V // 128  # 23 contraction tiles for mm1
KT2 = H // 128            # 32 contraction tiles for mm2
MT = B // 128             # 4 batch tiles

BIG = 64.0
MAGIC = 12582912.0        # 1.5 * 2^23: float32 round-to-int domain
BF16 = ml_dtypes.bfloat16

_cache = {}


def _build():
    import concourse.mybir as mybir
    import concourse.tile as tile
    from concourse import bacc

    fp32 = mybir.dt.float32
    bf16 = mybir.dt.bfloat16
    Alu = mybir.AluOpType
    Act = mybir.ActivationFunctionType

    nc = bacc.Bacc("TRN2", target_bir_lowering=False, debug=False,
                   num_devices=NCORES)

    # ---- per-core inputs ----
    xt = nc.dram_tensor("xt", [KT1, 128, B], bf16, kind="ExternalInput")
    w1h = nc.dram_tensor("w1h", [KT1, 128, HS], bf16, kind="ExternalInput")
    w1l = nc.dram_tensor("w1l", [KT1, 128, HS], bf16, kind="ExternalInput")
    b1s = nc.dram_tensor("b1s", [HM, 128], fp32, kind="ExternalInput")
    # W2 pre-tiled on host: per j-tile one contiguous [128, 8*CC] block
    # holding (hi,lo) x 4 column chunks -> one big DMA per contraction tile
    w2a = nc.dram_tensor("w2a", [KT2, 128, 2 * NCH * CC], bf16,
                         kind="ExternalInput")
    b2r = nc.dram_tensor("b2r", [128, CW], fp32, kind="ExternalInput")
    inpe = nc.dram_tensor("inpe", [MT, 128, EP * V], fp32, kind="ExternalInput")
    oute = nc.dram_tensor("oute", [MT, 128, EP * V], fp32, kind="ExternalOutput")

    # ---- constants (baked into the NEFF) ----
    iota_np = np.arange(V, dtype=np.float32)[None, :].repeat(128, 0)
    c_iota = nc.inline_tensor(np.ascontiguousarray(iota_np), name="c_iota")
    c_bi = nc.inline_tensor(np.ascontiguousarray(BIG - iota_np), name="c_bi")

    with tile.TileContext(nc) as tc:
        with (
            tc.tile_pool(name="persist", bufs=1) as persist,
            tc.tile_pool(name="hwork", bufs=2) as hwork,
            tc.tile_pool(name="w2s", bufs=4) as w2s,
            tc.tile_pool(name="ep", bufs=2) as ep,
            tc.tile_pool(name="small", bufs=2) as small,
            tc.tile_pool(name="ps", bufs=1, space="PSUM") as ps,
            tc.tile_pool(name="dram", bufs=1, space="DRAM") as dram,
        ):
            # ---------- dummy collective: absorbs start skew + comm setup ---
            warm_in = dram.tile([1, 16], fp32, tag="warm_in")
            warm_out = dram.tile([NCORES, 16], fp32, tag="warm_out",
                                 addr_space="Shared")
            nc.gpsimd.collective_compute(
                "AllGather", Alu.bypass,
                replica_groups=[list(range(NCORES))],
                ins=[warm_in.opt()], outs=[warm_out.opt()],
            )

            # ---------- constants to SBUF ----------
            iota_t = persist.tile([128, V], fp32, tag="iota")
            nc.sync.dma_start(iota_t[:], c_iota[:])
            cbi_t = persist.tile([128, V], fp32, tag="cbi")
            nc.sync.dma_start(cbi_t[:], c_bi[:])
            b2_t = persist.tile([128, CW], fp32, tag="b2")
            nc.sync.dma_start(b2_t[:], b2r[:])

            # ---------- load mm1 operands ----------
            xt_t, w1h_t, w1l_t = [], [], []
            for k in range(KT1):
                t = persist.tile([128, B], bf16, tag=f"xt{k}")
                nc.sync.dma_start(t[:], xt[k])
                xt_t.append(t)
                th = persist.tile([128, HS], bf16, tag=f"w1h{k}")
                nc.sync.dma_start(th[:], w1h[k])
                w1h_t.append(th)
                tl = persist.tile([128, HS], bf16, tag=f"w1l{k}")
                nc.sync.dma_start(tl[:], w1l[k])
                w1l_t.append(tl)
            b1_t = []
            for m in range(HM):
                t = persist.tile([128, 1], fp32, tag=f"b1{m}")
                nc.sync.dma_start(t[:], b1s[m].unsqueeze(1))
                b1_t.append(t)

            # token index per batch tile (dep-free, runs during mm1)
            t_tok = []
            for m in range(MT):
                it = ep.tile([128, EP * V], fp32, tag="inpe")
                nc.sync.dma_start(it[:], inpe[m])
                tk = persist.tile([128, EP], fp32, tag=f"tok{m}")
                tmp = ep.tile([128, EP, V], fp32, tag="tokmul")
                nc.vector.tensor_tensor(
                    tmp[:], it[:].rearrange("p (e v) -> p e v", v=V),
                    iota_t[:].unsqueeze(1).broadcast_to([128, EP, V]), Alu.mult)
                nc.vector.tensor_reduce(tk[:], tmp[:], axis=mybir.AxisListType.X,
                                        op=Alu.add)
                t_tok.append(tk)

            # ---------- per-chunk collective buffers ----------
            ag_in = [dram.tile([2, 128, B], bf16, tag=f"ag_in{m}",
                               name=f"ag_in{m}") for m in range(HM)]
            ag_out = [dram.tile([NCORES, 2, 128, B], bf16, tag=f"ag_out{m}",
                                name=f"ag_out{m}", addr_space="Shared")
                      for m in range(HM)]

            # ---------- phase 1: mm1 -> local hT tile, relu, split, gather --
            hhi_loc, hlo_loc = [], []
            for m in range(HM):
                acc = ps.tile([128, B], fp32, tag=f"p4_0_{m}", name=f"ps1_{m}")
                for k in range(KT1):
                    nc.tensor.matmul(acc[:], w1h_t[k][:, m * 128:(m + 1) * 128],
                                     xt_t[k][:], start=(k == 0), stop=False)
                for k in range(KT1):
                    nc.tensor.matmul(acc[:], w1l_t[k][:, m * 128:(m + 1) * 128],
                                     xt_t[k][:], start=False, stop=(k == KT1 - 1))
                hf = hwork.tile([128, B], fp32, tag="hf")
                nc.scalar.activation(hf[:], acc[:], Act.Relu, bias=b1_t[m], scale=1.0)
                hhi = hwork.tile([128, B], bf16, tag=f"hhi{m}", bufs=1)
                nc.vector.tensor_copy(hhi[:], hf[:])
                hlo = hwork.tile([128, B], bf16, tag=f"hlo{m}", bufs=1)
                nc.vector.tensor_sub(hlo[:], hf[:], hhi[:])
                hhi_loc.append(hhi)
                hlo_loc.append(hlo)
                nc.sync.dma_start(ag_in[m][0], hhi[:])
                nc.sync.dma_start(ag_in[m][1], hlo[:])
                nc.gpsimd.collective_compute(
                    "AllGather", Alu.bypass,
                    replica_groups=[list(range(NCORES))],
                    ins=[ag_in[m].opt()], outs=[ag_out[m].opt()],
                )

            # ---------- phase 3: gathered hT to SBUF (per chunk) ----------
            # reuse the xt/w1 tile slots -- same size, dead after mm1
            recycle = ([f"xt{k}" for k in range(KT1)]
                       + [f"w1h{k}" for k in range(KT1)]
                       + [f"w1l{k}" for k in range(KT1)])
            hth_t, htl_t = [None] * KT2, [None] * KT2
            for g in range(HM):
                for s in range(NCORES):
                    j = HM * s + g
                    th = persist.tile([128, B], bf16, tag=recycle[2 * j],
                                      name=f"hth{j}")
                    nc.sync.dma_start(th[:], ag_out[g][s, 0])
                    hth_t[j] = th
                    tl = persist.tile([128, B], bf16, tag=recycle[2 * j + 1],
                                      name=f"htl{j}")
                    nc.sync.dma_start(tl[:], ag_out[g][s, 1])
                    htl_t[j] = tl

            idx_all = [persist.tile([128, EP, 2], fp32, tag=f"idx{m}",
                                    name=f"idx{m}")
                       for m in range(MT)]

            # ---------- phase 4: mm2 --------------------------------------
            # m-tiles in pairs; each (j, m) loads the hT hi/lo stationaries
            # once and streams all 4 column chunks through them
            # (2 LDWEIGHTS per 12 matmuls). W2 is streamed once per m-pair.
            def mm2_epilogue(acc, nch, m):
                pv = acc[:].rearrange("p (i q c) -> p i q c", q=2, c=2 * V)
                bv = b2_t[:, nch * CC:(nch + 1) * CC].rearrange(
                    "p (i q c) -> p i q c", q=2, c=2 * V)
                netE = ep.tile([128, 4, 2 * V], fp32, tag="netE", name="netE")
                nc.vector.tensor_tensor(netE[:], pv[:, :, 0], bv[:, :, 0],
                                        Alu.add)
                ng = netE[:].rearrange("p i (s v) -> p i s v", v=V)
                gmax = ep.tile([128, 4, 2], fp32, tag="gmax", name="gmax")
                nc.vector.tensor_reduce(gmax[:], ng, axis=mybir.AxisListType.X,
                                        op=Alu.max)
                eq = ep.tile([128, 4, 2, V], fp32, tag="eq", name="eq")
                nc.vector.tensor_tensor(
                    eq[:], ng, gmax[:].unsqueeze(3).broadcast_to([128, 4, 2, V]),
                    Alu.is_ge)
                mt = ep.tile([128, 4, 2, V], fp32, tag="mt", name="mt")
                nc.vector.tensor_tensor(
                    mt[:], eq[:],
                    cbi_t[:].unsqueeze(1).unsqueeze(1).broadcast_to(
                        [128, 4, 2, V]), Alu.mult)
                tmax = ep.tile([128, 4, 2], fp32, tag="tmax", name="tmax")
                nc.vector.tensor_reduce(tmax[:], mt[:], axis=mybir.AxisListType.X,
                                        op=Alu.max)
                nc.vector.tensor_scalar(
                    idx_all[m][:, nch * 4:(nch + 1) * 4, :],
                    tmax[:], -1.0, BIG, Alu.mult, Alu.add)

            def mod23(dst_tag, src):
                d = small.tile([128, EP], fp32, tag=dst_tag + "_d",
                               name=dst_tag + "_d")
                nc.vector.tensor_scalar(d[:], src[:], 1.0 / 23.0, -0.49,
                                        Alu.mult, Alu.add)
                q = small.tile([128, EP], fp32, tag=dst_tag + "_q",
                               name=dst_tag + "_q")
                nc.vector.tensor_scalar(q[:], d[:], MAGIC, MAGIC,
                                        Alu.add, Alu.subtract)
                r = small.tile([128, EP], fp32, tag=dst_tag + "_r",
                               name=dst_tag + "_r")
                nc.vector.scalar_tensor_tensor(r[:], q[:], -23.0, src[:],
                                               Alu.mult, Alu.add)
                return r

            def flow_out(m):
                """argmax indices -> modular flow -> one-hot -> DRAM."""
                loc = idx_all[m][:, :, 0]
                scl = idx_all[m][:, :, 1]
                u0 = small.tile([128, EP], fp32, tag="u0", name="u0")
                nc.vector.scalar_tensor_tensor(u0[:], t_tok[m][:], 23.0, loc,
                                               Alu.add, Alu.subtract)
                geu = small.tile([128, EP], fp32, tag="geu", name="geu")
                nc.vector.tensor_single_scalar(geu[:], u0[:], 23.0, Alu.is_ge)
                u = small.tile([128, EP], fp32, tag="u", name="u")
                nc.vector.scalar_tensor_tensor(u[:], geu[:], -23.0, u0[:],
                                               Alu.mult, Alu.add)
                s2 = small.tile([128, EP], fp32, tag="s2", name="s2")
                nc.vector.tensor_tensor(s2[:], scl, scl, Alu.mult)
                s2m = mod23("s2m", s2)
                s4 = small.tile([128, EP], fp32, tag="s4", name="s4")
                nc.vector.tensor_tensor(s4[:], s2m[:], s2m[:], Alu.mult)
                s4m = mod23("s4m", s4)
                s8 = small.tile([128, EP], fp32, tag="s8", name="s8")
                nc.vector.tensor_tensor(s8[:], s4m[:], s4m[:], Alu.mult)
                s8m = mod23("s8m", s8)
                s16 = small.tile([128, EP], fp32, tag="s16", name="s16")
                nc.vector.tensor_tensor(s16[:], s8m[:], s8m[:], Alu.mult)
                s16m = mod23("s16m", s16)
                p1 = small.tile([128, EP], fp32, tag="p1", name="p1")
                nc.vector.tensor_tensor(p1[:], s16m[:], s4m[:], Alu.mult)
                p1m = mod23("p1m", p1)
                p2 = small.tile([128, EP], fp32, tag="p2", name="p2")
                nc.vector.tensor_tensor(p2[:], p1m[:], scl, Alu.mult)
                inv = mod23("inv", p2)
                wprod = small.tile([128, EP], fp32, tag="wprod", name="wprod")
                nc.vector.tensor_tensor(wprod[:], inv[:], u[:], Alu.mult)
                wm = mod23("wm", wprod)
                live = small.tile([128, EP], fp32, tag="live", name="live")
                nc.vector.tensor_single_scalar(live[:], inv[:], 0.5, Alu.is_ge)
                w1p = small.tile([128, EP], fp32, tag="w1p", name="w1p")
                nc.vector.tensor_single_scalar(w1p[:], wm[:], 1.0, Alu.add)
                w2p = small.tile([128, EP], fp32, tag="w2p", name="w2p")
                nc.vector.tensor_tensor(w2p[:], w1p[:], live[:], Alu.mult)
                wfin = small.tile([128, EP], fp32, tag="wfin", name="wfin")
                nc.vector.tensor_single_scalar(wfin[:], w2p[:], -1.0, Alu.add)
                oh = ep.tile([128, EP, V], fp32, tag="oh", name="oh")
                nc.vector.tensor_tensor(
                    oh[:], iota_t[:].unsqueeze(1).broadcast_to([128, EP, V]),
                    wfin[:].unsqueeze(2).broadcast_to([128, EP, V]), Alu.is_equal)
                nc.sync.dma_start(oute[m], oh[:].rearrange("p e v -> p (e v)"))

            for mp in range(MT // 2):          # m-pairs (0,1), (2,3)
                ms = (2 * mp, 2 * mp + 1)
                accs = {(mi, nch): ps.tile([128, CC], fp32,
                                           tag=f"p4_{mi}_{nch}",
                                           name=f"p4_{mp}_{mi}_{nch}")
                        for mi in range(2) for nch in range(NCH)}
                for g in range(HM):
                    for s in range(NCORES):
                        j = HM * s + g
                        first = (g == 0 and s == 0)
                        last = (g == HM - 1 and s == NCORES - 1)
                        w2t = w2s.tile([128, 2 * NCH * CC], bf16, tag="w2t",
                                       name="w2t")
                        nc.sync.dma_start(w2t[:], w2a[j])
                        rh = [w2t[:, (2 * n) * CC:(2 * n + 1) * CC]
                              for n in range(NCH)]
                        rl = [w2t[:, (2 * n + 1) * CC:(2 * n + 2) * CC]
                              for n in range(NCH)]
                        for mi, m in enumerate(ms):
                            lh = hth_t[j][:, m * 128:(m + 1) * 128]
                            ll = htl_t[j][:, m * 128:(m + 1) * 128]
                            for nch in range(NCH):
                                a = accs[(mi, nch)]
                                nc.tensor.matmul(a[:], lh, rh[nch],
                                                 start=first, stop=False)
                                nc.tensor.matmul(a[:], lh, rl[nch],
                                                 start=False, stop=False)
                            for nch in range(NCH):
                                a = accs[(mi, nch)]
                                nc.tensor.matmul(a[:], ll, rh[nch],
                                                 start=False, stop=last)
                for mi, m in enumerate(ms):
                    for nch in range(NCH):
                        mm2_epilogue(accs[(mi, nch)], nch, m)
                for m in ms:
                    flow_out(m)

    nc.compile()
    return nc


def _split_bf16(a):
    hi = a.astype(BF16)
    lo = (a - hi.astype(np.float32)).astype(BF16)
    return hi, lo


def kernel(inputs, mask, W1, b1, W2, b2):
    from concourse.bass_utils import run_bass_kernel_spmd

    if "nc" not in _cache:
        _cache["nc"] = _build()
    nc = _cache["nc"]

    inputs = np.asarray(inputs, np.float32)
    mask = np.asarray(mask, np.float32)
    W1 = np.asarray(W1, np.float32)
    b1 = np.asarray(b1, np.float32)
    W2 = np.asarray(W2, np.float32)
    b2 = np.asarray(b2, np.float32)

    masked = inputs * mask[None, :, :]                    # [B, L, V]
    x_odd = masked[:, 1::2, :].reshape(B, (L // 2) * V)   # [512, 2944]
    xt_np = np.ascontiguousarray(x_odd.T.reshape(KT1, 128, B)).astype(BF16)
    W1_odd = W1.reshape(L, V, H)[1::2].reshape((L // 2) * V, H)

    in_maps = []
    for k in range(NCORES):
        w1s = W1_odd[:, k * HS:(k + 1) * HS]
        w1hi, w1lo = _split_bf16(w1s)
        w2sl = W2[:, k * CW:(k + 1) * CW]
        w2hi, w2lo = _split_bf16(w2sl)
        # interleave (hi,lo) per column chunk: [j, 128, 2*NCH*CC] contiguous
        w2hi = w2hi.reshape(KT2, 128, NCH, CC)
        w2lo = w2lo.reshape(KT2, 128, NCH, CC)
        w2all = np.empty((KT2, 128, 2 * NCH, CC), dtype=BF16)
        w2all[:, :, 0::2] = w2hi
        w2all[:, :, 1::2] = w2lo
        w2all = np.ascontiguousarray(w2all.reshape(KT2, 128, 2 * NCH * CC))
        b2s = b2[k * CW:(k + 1) * CW]
        cols = slice(32 * k, 32 * k + 32, 2)
        inpe = inputs[:, cols, :].reshape(MT, 128, EP * V)
        in_maps.append({
            "xt": xt_np,
            "w1h": np.ascontiguousarray(w1hi.reshape(KT1, 128, HS)),
            "w1l": np.ascontiguousarray(w1lo.reshape(KT1, 128, HS)),
            "b1s": np.ascontiguousarray(b1[k * HS:(k + 1) * HS].reshape(-1, 128)),
            "w2a": w2all,
            "b2r": np.ascontiguousarray(np.broadcast_to(b2s, (128, CW))),
            "inpe": np.ascontiguousarray(inpe),
        })

    res = run_bass_kernel_spmd(nc, in_maps, core_ids=list(range(NCORES)))
    _cache["last_result"] = res

    out = np.empty((B, L, V), np.float32)
    out[:, 1::2, :] = masked[:, 1::2, :]
    for k in range(NCORES):
        oe = res.results[k]["oute"].reshape(MT, 128, EP, V)
        out[:, 32 * k:32 * k + 32:2, :] = oe.reshape(B, EP, V)
    return out


# revision 13
# speedup vs baseline: 1.7947x; 1.5547x over previous
"""DiscreteBipartiteFlow forward on 8 Trainium2 NeuronCores.

Math (forward pass only):
  masked = mask * inputs                      (mask = 1 at odd l, 0 at even l)
  h   = relu(masked.reshape(B, L*V) @ W1 + b1)
  net = (h @ W2 + b2).reshape(B, L, 2V)
  loc, scale = argmax one-hots of net[..., :V], net[..., V:]
  out[odd l]  = inputs
  out[even l] = onehot((inv(scale) * ((tok - loc) mod V)) mod V), or 0 if scale==0

st_one_hot_argmax's forward value is exactly the hard one-hot (soft terms
cancel), so the post-MLP flow is pure index arithmetic mod 23.

Sharding (8 cores):
  mm1: tensor-parallel over hidden. Core k computes hT[512k:512k+512, :] from
       the odd-position one-hot rows only (mask zeroes even rows; 2944 of 5888
       W1 rows ever contribute).
  all-gather: h, split into bf16 hi+lo, gathered in 4 chunks (one per local
       128-row tile) so communication pipelines under mm2; a zero-dep dummy
       collective at t=0 absorbs cross-core start skew + comm setup.
  mm2: tensor-parallel over output columns. Core k owns positions
       l in [32k, 32k+32) = 1472 columns of W2. The contraction loop is
       grouped by gather chunk (j % 4) so each chunk's matmuls start as soon
       as its gather lands.
  epilogue: per-core argmax + modular flow for its 16 even positions;
       host interleaves position slices and passes odd positions through.

Precision: matmuls run as bf16 hi/lo split passes (x one-hot is exact in
bf16, so mm1 = 2 passes over W1{hi,lo}; mm2 = 3 passes hh+hl+lh) with fp32
PSUM accumulation -> ~2^-18 operand error, fp32-grade argmax fidelity, at
1 cycle/row TensorE throughput (vs 4 cycles/row for native fp32).
"""

import numpy as np
import ml_dtypes

B, L, V = 512, 256, 23
H = 4096
NCORES = 8
HS = H // NCORES          # 512  hidden shard
HM = HS // 128            # 4    local hidden tiles
PS = L // NCORES          # 32   positions per core
EP = PS // 2              # 16   even positions per core
CW = PS * 2 * V           # 1472 net columns per core (incl. unused odd)
CE = EP * 2 * V           # 736  even-position net columns (the used ones)
NCH = 2                   # column chunks for mm2
CC = CE // NCH            # 368  columns per chunk (8 even positions)
KT1 = (L // 2) * V // 128  # 23 contraction tiles for mm1
KT2 = H // 128            # 32 contraction tiles for mm2
MT = B // 128             # 4 batch tiles

BIG = 64.0
MAGIC = 12582912.0        # 1.5 * 2^23: float32 round-to-int domain
BF16 = ml_dtypes.bfloat16

_cache = {}


def _build():
    import concourse.mybir as mybir
    import concourse.tile as tile
    from concourse import bacc

    fp32 = mybir.dt.float32
    bf16 = mybir.dt.bfloat16
    Alu = mybir.AluOpType
    Act = mybir.ActivationFunctionType

    nc = bacc.Bacc("TRN2", target_bir_lowering=False, debug=False,
                   num_devices=NCORES)

    # ---- per-core inputs ----
    xt = nc.dram_tensor("xt", [KT1, 128, B], bf16, kind="ExternalInput")
    w1h = nc.dram_tensor("w1h", [KT1, 128, HS], bf16, kind="ExternalInput")
    w1l = nc.dram_tensor("w1l", [KT1, 128, HS], bf16, kind="ExternalInput")
    b1s = nc.dram_tensor("b1s", [HM, 128], fp32, kind="ExternalInput")
    # W2 pre-tiled on host: per j-tile one contiguous [128, 8*CC] block
    # holding (hi,lo) x 4 column chunks -> one big DMA per contraction tile
    w2a = nc.dram_tensor("w2a", [KT2, 128, 2 * NCH * CC], bf16,
                         kind="ExternalInput")
    b2r = nc.dram_tensor("b2r", [128, CE], fp32, kind="ExternalInput")
    inpe = nc.dram_tensor("inpe", [MT, 128, EP * V], fp32, kind="ExternalInput")
    oute = nc.dram_tensor("oute", [MT, 128, EP * V], fp32, kind="ExternalOutput")

    # ---- constants (baked into the NEFF) ----
    iota_np = np.arange(V, dtype=np.float32)[None, :].repeat(128, 0)
    c_iota = nc.inline_tensor(np.ascontiguousarray(iota_np), name="c_iota")
    c_bi = nc.inline_tensor(np.ascontiguousarray(BIG - iota_np), name="c_bi")

    with tile.TileContext(nc) as tc:
        with (
            tc.tile_pool(name="persist", bufs=1) as persist,
            tc.tile_pool(name="hwork", bufs=2) as hwork,
            tc.tile_pool(name="w2s", bufs=4) as w2s,
            tc.tile_pool(name="ep", bufs=2) as ep,
            tc.tile_pool(name="small", bufs=2) as small,
            tc.tile_pool(name="ps", bufs=1, space="PSUM") as ps,
            tc.tile_pool(name="dram", bufs=1, space="DRAM") as dram,
        ):
            # ---------- dummy collective: absorbs start skew + comm setup ---
            warm_in = dram.tile([1, 16], fp32, tag="warm_in")
            warm_out = dram.tile([NCORES, 16], fp32, tag="warm_out",
                                 addr_space="Shared")
            nc.gpsimd.collective_compute(
                "AllGather", Alu.bypass,
                replica_groups=[list(range(NCORES))],
                ins=[warm_in.opt()], outs=[warm_out.opt()],
            )

            # ---------- constants to SBUF ----------
            iota_t = persist.tile([128, V], fp32, tag="iota")
            nc.sync.dma_start(iota_t[:], c_iota[:])
            cbi_t = persist.tile([128, V], fp32, tag="cbi")
            nc.sync.dma_start(cbi_t[:], c_bi[:])
            b2_t = persist.tile([128, CE], fp32, tag="b2")
            nc.sync.dma_start(b2_t[:], b2r[:])

            # ---------- load mm1 operands ----------
            xt_t, w1h_t, w1l_t = [], [], []
            for k in range(KT1):
                t = persist.tile([128, B], bf16, tag=f"xt{k}")
                nc.sync.dma_start(t[:], xt[k])
                xt_t.append(t)
                th = persist.tile([128, HS], bf16, tag=f"w1h{k}")
                nc.sync.dma_start(th[:], w1h[k])
                w1h_t.append(th)
                tl = persist.tile([128, HS], bf16, tag=f"w1l{k}")
                nc.sync.dma_start(tl[:], w1l[k])
                w1l_t.append(tl)
            b1_t = []
            for m in range(HM):
                t = persist.tile([128, 1], fp32, tag=f"b1{m}")
                nc.sync.dma_start(t[:], b1s[m].unsqueeze(1))
                b1_t.append(t)

            # token index per batch tile (dep-free, runs during mm1)
            t_tok = []
            for m in range(MT):
                it = ep.tile([128, EP * V], fp32, tag="inpe")
                nc.sync.dma_start(it[:], inpe[m])
                tk = persist.tile([128, EP], fp32, tag=f"tok{m}")
                tmp = ep.tile([128, EP, V], fp32, tag="tokmul")
                nc.vector.tensor_tensor(
                    tmp[:], it[:].rearrange("p (e v) -> p e v", v=V),
                    iota_t[:].unsqueeze(1).broadcast_to([128, EP, V]), Alu.mult)
                nc.vector.tensor_reduce(tk[:], tmp[:], axis=mybir.AxisListType.X,
                                        op=Alu.add)
                t_tok.append(tk)

            # ---------- per-chunk collective buffers ----------
            ag_in = [dram.tile([2, 128, B], bf16, tag=f"ag_in{m}",
                               name=f"ag_in{m}") for m in range(HM)]
            ag_out = [dram.tile([NCORES, 2, 128, B], bf16, tag=f"ag_out{m}",
                                name=f"ag_out{m}", addr_space="Shared")
                      for m in range(HM)]

            # ---------- phase 1: mm1 -> local hT tile, relu, split, gather --
            hhi_loc, hlo_loc = [], []
            for m in range(HM):
                acc = ps.tile([128, B], fp32, tag=f"p4_{m}_0", name=f"ps1_{m}")
                for k in range(KT1):
                    nc.tensor.matmul(acc[:], w1h_t[k][:, m * 128:(m + 1) * 128],
                                     xt_t[k][:], start=(k == 0), stop=False)
                for k in range(KT1):
                    nc.tensor.matmul(acc[:], w1l_t[k][:, m * 128:(m + 1) * 128],
                                     xt_t[k][:], start=False, stop=(k == KT1 - 1))
                hf = hwork.tile([128, B], fp32, tag="hf")
                nc.scalar.activation(hf[:], acc[:], Act.Relu, bias=b1_t[m], scale=1.0)
                hhi = hwork.tile([128, B], bf16, tag=f"hhi{m}", bufs=1)
                nc.vector.tensor_copy(hhi[:], hf[:])
                hlo = hwork.tile([128, B], bf16, tag=f"hlo{m}", bufs=1)
                nc.vector.tensor_sub(hlo[:], hf[:], hhi[:])
                hhi_loc.append(hhi)
                hlo_loc.append(hlo)
                nc.sync.dma_start(ag_in[m][0], hhi[:])
                nc.sync.dma_start(ag_in[m][1], hlo[:])
                nc.gpsimd.collective_compute(
                    "AllGather", Alu.bypass,
                    replica_groups=[list(range(NCORES))],
                    ins=[ag_in[m].opt()], outs=[ag_out[m].opt()],
                )

            # ---------- phase 3: gathered hT to SBUF (per chunk) ----------
            # reuse the xt/w1 tile slots -- same size, dead after mm1
            recycle = ([f"xt{k}" for k in range(KT1)]
                       + [f"w1h{k}" for k in range(KT1)]
                       + [f"w1l{k}" for k in range(KT1)])
            hth_t, htl_t = [None] * KT2, [None] * KT2
            for g in range(HM):
                for s in range(NCORES):
                    j = HM * s + g
                    th = persist.tile([128, B], bf16, tag=recycle[2 * j],
                                      name=f"hth{j}")
                    nc.sync.dma_start(th[:], ag_out[g][s, 0])
                    hth_t[j] = th
                    tl = persist.tile([128, B], bf16, tag=recycle[2 * j + 1],
                                      name=f"htl{j}")
                    nc.sync.dma_start(tl[:], ag_out[g][s, 1])
                    htl_t[j] = tl

            idx_all = [persist.tile([128, EP, 2], fp32, tag=f"idx{m}",
                                    name=f"idx{m}")
                       for m in range(MT)]

            # ---------- phase 4: mm2 --------------------------------------
            # m-tiles in pairs; each (j, m) loads the hT hi/lo stationaries
            # once and streams all 4 column chunks through them
            # (2 LDWEIGHTS per 12 matmuls). W2 is streamed once per m-pair.
            def mm2_epilogue(acc, nch, m):
                # acc: [128, 368] = 8 even positions x (loc|scale) x 23
                bv = b2_t[:, nch * CC:(nch + 1) * CC]
                netE = ep.tile([128, CC], fp32, tag="netE", name="netE")
                nc.vector.tensor_tensor(netE[:], acc[:], bv, Alu.add)
                ng = netE[:].rearrange("p (i s v) -> p i s v", s=2, v=V)
                gmax = ep.tile([128, 8, 2], fp32, tag="gmax", name="gmax")
                nc.vector.tensor_reduce(gmax[:], ng, axis=mybir.AxisListType.X,
                                        op=Alu.max)
                eq = ep.tile([128, 8, 2, V], fp32, tag="eq", name="eq")
                nc.vector.tensor_tensor(
                    eq[:], ng, gmax[:].unsqueeze(3).broadcast_to([128, 8, 2, V]),
                    Alu.is_ge)
                mt = ep.tile([128, 8, 2, V], fp32, tag="mt", name="mt")
                nc.vector.tensor_tensor(
                    mt[:], eq[:],
                    cbi_t[:].unsqueeze(1).unsqueeze(1).broadcast_to(
                        [128, 8, 2, V]), Alu.mult)
                tmax = ep.tile([128, 8, 2], fp32, tag="tmax", name="tmax")
                nc.vector.tensor_reduce(tmax[:], mt[:], axis=mybir.AxisListType.X,
                                        op=Alu.max)
                nc.vector.tensor_scalar(
                    idx_all[m][:, nch * 8:(nch + 1) * 8, :],
                    tmax[:], -1.0, BIG, Alu.mult, Alu.add)

            def mod23(dst_tag, src):
                d = small.tile([128, EP], fp32, tag=dst_tag + "_d",
                               name=dst_tag + "_d")
                nc.vector.tensor_scalar(d[:], src[:], 1.0 / 23.0, -0.49,
                                        Alu.mult, Alu.add)
                q = small.tile([128, EP], fp32, tag=dst_tag + "_q",
                               name=dst_tag + "_q")
                nc.vector.tensor_scalar(q[:], d[:], MAGIC, MAGIC,
                                        Alu.add, Alu.subtract)
                r = small.tile([128, EP], fp32, tag=dst_tag + "_r",
                               name=dst_tag + "_r")
                nc.vector.scalar_tensor_tensor(r[:], q[:], -23.0, src[:],
                                               Alu.mult, Alu.add)
                return r

            def flow_out(m):
                """argmax indices -> modular flow -> one-hot -> DRAM."""
                loc = idx_all[m][:, :, 0]
                scl = idx_all[m][:, :, 1]
                u0 = small.tile([128, EP], fp32, tag="u0", name="u0")
                nc.vector.scalar_tensor_tensor(u0[:], t_tok[m][:], 23.0, loc,
                                               Alu.add, Alu.subtract)
                geu = small.tile([128, EP], fp32, tag="geu", name="geu")
                nc.vector.tensor_single_scalar(geu[:], u0[:], 23.0, Alu.is_ge)
                u = small.tile([128, EP], fp32, tag="u", name="u")
                nc.vector.scalar_tensor_tensor(u[:], geu[:], -23.0, u0[:],
                                               Alu.mult, Alu.add)
                s2 = small.tile([128, EP], fp32, tag="s2", name="s2")
                nc.vector.tensor_tensor(s2[:], scl, scl, Alu.mult)
                s2m = mod23("s2m", s2)
                s4 = small.tile([128, EP], fp32, tag="s4", name="s4")
                nc.vector.tensor_tensor(s4[:], s2m[:], s2m[:], Alu.mult)
                s4m = mod23("s4m", s4)
                s8 = small.tile([128, EP], fp32, tag="s8", name="s8")
                nc.vector.tensor_tensor(s8[:], s4m[:], s4m[:], Alu.mult)
                s8m = mod23("s8m", s8)
                s16 = small.tile([128, EP], fp32, tag="s16", name="s16")
                nc.vector.tensor_tensor(s16[:], s8m[:], s8m[:], Alu.mult)
                s16m = mod23("s16m", s16)
                p1 = small.tile([128, EP], fp32, tag="p1", name="p1")
                nc.vector.tensor_tensor(p1[:], s16m[:], s4m[:], Alu.mult)
                p1m = mod23("p1m", p1)
                p2 = small.tile([128, EP], fp32, tag="p2", name="p2")
                nc.vector.tensor_tensor(p2[:], p1m[:], scl, Alu.mult)
                inv = mod23("inv", p2)
                wprod = small.tile([128, EP], fp32, tag="wprod", name="wprod")
                nc.vector.tensor_tensor(wprod[:], inv[:], u[:], Alu.mult)
                wm = mod23("wm", wprod)
                live = small.tile([128, EP], fp32, tag="live", name="live")
                nc.vector.tensor_single_scalar(live[:], inv[:], 0.5, Alu.is_ge)
                w1p = small.tile([128, EP], fp32, tag="w1p", name="w1p")
                nc.vector.tensor_single_scalar(w1p[:], wm[:], 1.0, Alu.add)
                w2p = small.tile([128, EP], fp32, tag="w2p", name="w2p")
                nc.vector.tensor_tensor(w2p[:], w1p[:], live[:], Alu.mult)
                wfin = small.tile([128, EP], fp32, tag="wfin", name="wfin")
                nc.vector.tensor_single_scalar(wfin[:], w2p[:], -1.0, Alu.add)
                oh = ep.tile([128, EP, V], fp32, tag="oh", name="oh")
                nc.vector.tensor_tensor(
                    oh[:], iota_t[:].unsqueeze(1).broadcast_to([128, EP, V]),
                    wfin[:].unsqueeze(2).broadcast_to([128, EP, V]), Alu.is_equal)
                nc.sync.dma_start(oute[m], oh[:].rearrange("p e v -> p (e v)"))

            # single j-sweep; 2 column chunks x 4 m-tiles = 8 PSUM banks;
            # each (j, m) loads hT hi/lo once for 6 matmuls
            accs = {(m, nch): ps.tile([128, CC], fp32, tag=f"p4_{m}_{nch}",
                                      name=f"p4_{m}_{nch}")
                    for m in range(MT) for nch in range(NCH)}
            for g in range(HM):
                for s in range(NCORES):
                    j = HM * s + g
                    first = (g == 0 and s == 0)
                    last = (g == HM - 1 and s == NCORES - 1)
                    w2t = w2s.tile([128, 2 * NCH * CC], bf16, tag="w2t",
                                   name="w2t")
                    nc.sync.dma_start(w2t[:], w2a[j])
                    rh = [w2t[:, (2 * n) * CC:(2 * n + 1) * CC]
                          for n in range(NCH)]
                    rl = [w2t[:, (2 * n + 1) * CC:(2 * n + 2) * CC]
                          for n in range(NCH)]
                    for m in range(MT):
                        lh = hth_t[j][:, m * 128:(m + 1) * 128]
                        ll = htl_t[j][:, m * 128:(m + 1) * 128]
                        for nch in range(NCH):
                            a = accs[(m, nch)]
                            nc.tensor.matmul(a[:], lh, rh[nch],
                                             start=first, stop=False)
                            nc.tensor.matmul(a[:], lh, rl[nch],
                                             start=False, stop=False)
                        for nch in range(NCH):
                            a = accs[(m, nch)]
                            nc.tensor.matmul(a[:], ll, rh[nch],
                                             start=False, stop=last)
            for m in range(MT):
                for nch in range(NCH):
                    mm2_epilogue(accs[(m, nch)], nch, m)
                flow_out(m)

    nc.compile()
    return nc


def _split_bf16(a):
    hi = a.astype(BF16)
    lo = (a - hi.astype(np.float32)).astype(BF16)
    return hi, lo


def kernel(inputs, mask, W1, b1, W2, b2):
    from concourse.bass_utils import run_bass_kernel_spmd

    if "nc" not in _cache:
        _cache["nc"] = _build()
    nc = _cache["nc"]

    inputs = np.asarray(inputs, np.float32)
    mask = np.asarray(mask, np.float32)
    W1 = np.asarray(W1, np.float32)
    b1 = np.asarray(b1, np.float32)
    W2 = np.asarray(W2, np.float32)
    b2 = np.asarray(b2, np.float32)

    masked = inputs * mask[None, :, :]                    # [B, L, V]
    x_odd = masked[:, 1::2, :].reshape(B, (L // 2) * V)   # [512, 2944]
    xt_np = np.ascontiguousarray(x_odd.T.reshape(KT1, 128, B)).astype(BF16)
    W1_odd = W1.reshape(L, V, H)[1::2].reshape((L // 2) * V, H)

    in_maps = []
    for k in range(NCORES):
        w1s = W1_odd[:, k * HS:(k + 1) * HS]
        w1hi, w1lo = _split_bf16(w1s)
        # odd-position net columns are multiplied by (1-mask)=0 downstream:
        # only the 736 even-position columns of this core's W2 slice matter
        w2sl = W2[:, k * CW:(k + 1) * CW].reshape(H, PS, 2 * V)[:, 0::2, :]
        w2sl = w2sl.reshape(H, CE)
        w2hi, w2lo = _split_bf16(w2sl)
        # interleave (hi,lo) per column chunk: [j, 128, 2*NCH*CC] contiguous
        w2hi = w2hi.reshape(KT2, 128, NCH, CC)
        w2lo = w2lo.reshape(KT2, 128, NCH, CC)
        w2all = np.empty((KT2, 128, 2 * NCH, CC), dtype=BF16)
        w2all[:, :, 0::2] = w2hi
        w2all[:, :, 1::2] = w2lo
        w2all = np.ascontiguousarray(w2all.reshape(KT2, 128, 2 * NCH * CC))
        b2s = b2[k * CW:(k + 1) * CW].reshape(PS, 2 * V)[0::2].reshape(CE)
        cols = slice(32 * k, 32 * k + 32, 2)
        inpe = inputs[:, cols, :].reshape(MT, 128, EP * V)
        in_maps.append({
            "xt": xt_np,
            "w1h": np.ascontiguousarray(w1hi.reshape(KT1, 128, HS)),
            "w1l": np.ascontiguousarray(w1lo.reshape(KT1, 128, HS)),
            "b1s": np.ascontiguousarray(b1[k * HS:(k + 1) * HS].reshape(-1, 128)),
            "w2a": w2all,
            "b2r": np.ascontiguousarray(np.broadcast_to(b2s, (128, CE))),
            "inpe": np.ascontiguousarray(inpe),
        })

    res = run_bass_kernel_spmd(nc, in_maps, core_ids=list(range(NCORES)))
    _cache["last_result"] = res

    out = np.empty((B, L, V), np.float32)
    out[:, 1::2, :] = masked[:, 1::2, :]
    for k in range(NCORES):
        oe = res.results[k]["oute"].reshape(MT, 128, EP, V)
        out[:, 32 * k:32 * k + 32:2, :] = oe.reshape(B, EP, V)
    return out
